# revision 11
# baseline (speedup 1.0000x reference)
"""GCN layer v5 on 8 Trainium2 NeuronCores.

v3 + range-major gather packing:
- Edges ordered (range, sb, dloc) so gather calls span sb boundaries within
  a range (same in_ap): ~240 nearly-full 896-row calls instead of 319
  (the SWDGE ring caps calls at 64 descs = 896 rows, and per-call fixed
  cost ~1.1us on the Pool engine is the kernel's bottleneck).
- Gathers land in fixed 3584-row et segment tiles (4 calls each).
- psT [128,512] PSUM partials per (sb, range) group are accumulated into
  25 persistent SBUF agg tiles (Act zero-inits psT, DVE adds partials);
  after the last range sweep each sb runs the W-matmul + norm_dst output.
- One-hot S via broadcast tensor_tensor (is_equal) per group, fp16;
  feat pre-scaled by norm_src on host; norm_dst in Act output copies.
"""
import numpy as np

N_NODES = 100000
N_EDGES = 1600000
F = 128
N_CORES = 8
OWN = 12544
SB = 512
NSB = 25
RANGE = 32768
NRANGES = 4
MAX_CALL = 896      # dma_gather single-call cap (ring = 64 descs incl sem)
SEG_CALLS = 2       # gather calls per et segment tile
SEG_ROWS = MAX_CALL * SEG_CALLS   # 3584 rows = 28 chunks
PIECE = 24          # one-hot entries per DVE instruction


def _install_walrus_passes():
    import concourse.bass_utils as bu

    def patched(tmpdir, inp="bir.json", outp="file.neff", arch=None, *, dve_root=None):
        from pathlib import Path
        cmd = [
            bu.get_walrus_driver(),
            "--pass",
            "birverifier,dynamic_dma_scan,runtime_memory_reservation,"
            "dynamic_dma_setup,lower_act,lower_dve,lower_ap_offset,"
            "codegen,neff_packager",
            "-i", inp,
            "--neff-output-filename", outp,
            "--enable-birsim=true",
            "--mem-mode=physical",
            "--policy=0",
            "--enable-ldw-opt=false",
            "--assign-static-dmas-to-sp=false",
            "--dram-page-size=256",
            "--enable-neff-debug-info=true",
            "--jobs", "8",
            "--dynamic-dma-scratch-size-per-partition=16384",
            *bu.get_walrus_args(
                bu.get_bir_arch(tmpdir, inp) if arch is None else arch,
                tmpdir, dve_root=dve_root,
            ),
        ]
        result = bu.run_command(cmd, cwd=tmpdir)
        if result is not None:
            (Path(tmpdir) / "log.txt").write_text(result.stdout)
        return f"{tmpdir}/{outp}"

    bu.bir_verify_and_optimise = patched


def _pack_idx_wrap(idx_i16: np.ndarray, cap: int) -> np.ndarray:
    w = np.zeros((16, cap // 16), np.int16)
    j = np.arange(len(idx_i16))
    w[j % 16, j // 16] = idx_i16
    return np.tile(w, (8, 1))


def _preprocess(src: np.ndarray, dst: np.ndarray):
    """Returns (prog, per_core, norm_src, norm_dst_percore)."""
    src = np.asarray(src).astype(np.int64)
    dst = np.asarray(dst).astype(np.int64)

    out_deg = np.bincount(src, minlength=N_NODES).astype(np.float32)
    in_deg = np.bincount(dst, minlength=N_NODES).astype(np.float32)
    norm_src = 1.0 / np.sqrt(np.clip(out_deg, 1.0, None))
    norm_dst = 1.0 / np.sqrt(np.clip(in_deg, 1.0, None))

    core = np.minimum(dst // OWN, N_CORES - 1)
    dst_local = dst - core * OWN
    sb = dst_local // SB
    dloc = dst_local - sb * SB
    rng = src // RANGE

    sizes = np.zeros((N_CORES, NSB, NRANGES), np.int64)
    np.add.at(sizes, (core, sb, rng), 1)
    gmax = sizes.max(axis=0)
    gpad = ((gmax + 127) // 128) * 128          # [NSB, NRANGES]

    # range-major chunk grid: chunks of groups (s, r) laid out r-major
    group_chunk0 = {}
    total_chunks = 0
    range_chunk0 = []
    for r in range(NRANGES):
        range_chunk0.append(total_chunks)
        for s in range(NSB):
            group_chunk0[(s, r)] = total_chunks
            total_chunks += int(gpad[s, r]) // 128
    range_chunk0.append(total_chunks)

    # gather call plan: per range, calls of <=896 rows inside 3584-row
    # (28-chunk) segments; (r, seg_idx, chunk_offset, n_idx)
    plan = []
    seg_of_range = []      # per range: number of segments
    for r in range(NRANGES):
        c0 = range_chunk0[r]
        c1 = range_chunk0[r + 1]
        nch_r = c1 - c0
        nseg = (nch_r + SEG_ROWS // 128 - 1) // (SEG_ROWS // 128)
        seg_of_range.append(nseg)
        for g in range(nseg):
            sc0 = c0 + g * (SEG_ROWS // 128)
            sc1 = min(sc0 + SEG_ROWS // 128, c1)
            rows = (sc1 - sc0) * 128
            off = 0
            while off < rows:
                take = min(MAX_CALL, rows - off)
                plan.append((r, g, sc0 + off // 128, take))
                off += take
    idx_cols = total_chunks * 8

    # per-core padded streams in range-major order
    dlmin = np.full(total_chunks, SB, np.int64)
    dlmax = np.full(total_chunks, -1, np.int64)
    core_streams = []
    for k in range(N_CORES):
        m = core == k
        e_sb, e_rng = sb[m], rng[m]
        e_src, e_dl = src[m], dloc[m]
        order = np.lexsort((e_dl, e_sb, e_rng))
        e_sb, e_rng = e_sb[order], e_rng[order]
        e_src, e_dl = e_src[order], e_dl[order]

        idx_stream = np.zeros(total_chunks * 128, np.int16)
        dl_stream = np.full(total_chunks * 128, -1, np.int64)  # -1 = pad

        gsizes = np.zeros((NSB, NRANGES), np.int64)
        np.add.at(gsizes, (e_sb, e_rng), 1)
        acc = 0
        for r in range(NRANGES):
            for s in range(NSB):
                n_real = int(gsizes[s, r])
                if n_real == 0:
                    continue
                p0 = group_chunk0[(s, r)] * 128
                sl = slice(p0, p0 + n_real)
                idx_stream[sl] = (e_src[acc:acc + n_real] - r * RANGE).astype(np.int16)
                dl_stream[sl] = e_dl[acc:acc + n_real]
                acc += n_real
        core_streams.append((idx_stream, dl_stream))

        dl2 = dl_stream.reshape(total_chunks, 128)
        v2 = dl2 >= 0
        has = v2.any(axis=1)
        cmin = np.where(has, np.where(v2, dl2, SB).min(axis=1), SB)
        cmax = np.where(has, np.where(v2, dl2, -1).max(axis=1), -1)
        dlmin = np.minimum(dlmin, cmin)
        dlmax = np.maximum(dlmax, cmax)

    # entries: per chunk, ceil(span/128) windows starting at min(dlmin, 384)
    has_any = dlmax >= 0
    b0 = np.minimum(np.where(has_any, dlmin, 0), SB - 128)
    kcnt = np.where(has_any, (dlmax - b0) // 128 + 1, 1)
    entry_chunk = []
    entry_base = []
    entry_off = np.zeros(total_chunks + 1, np.int64)
    for c in range(total_chunks):
        entry_off[c] = len(entry_chunk)
        for k in range(int(kcnt[c])):
            entry_chunk.append(c)
            entry_base.append(min(int(b0[c]) + 128 * k, SB - 128))
        entry_off[total_chunks] = len(entry_chunk)
    entry_chunk = np.array(entry_chunk, np.int64)
    entry_base = np.array(entry_base, np.int64)
    total_entries = len(entry_chunk)

    # per-group chunk/entry ranges
    group_ranges = {}      # (s, r) -> (gc0, gc1, ge0, ge1)
    for r in range(NRANGES):
        for s in range(NSB):
            gc0 = group_chunk0[(s, r)]
            gc1 = gc0 + int(gpad[s, r]) // 128
            group_ranges[(s, r)] = (gc0, gc1,
                                    int(entry_off[gc0]), int(entry_off[gc1]))
    range_entry0 = [int(entry_off[range_chunk0[r]]) for r in range(NRANGES)]
    range_entry0.append(total_entries)

    # per-core dl buffers [128, total_entries] fp16 (999 = no match)
    per_core = []
    for k in range(N_CORES):
        idx_stream, dl_stream = core_streams[k]
        pos = np.arange(total_chunks * 128)
        c_of = pos // 128
        p_of = pos % 128
        valid = dl_stream >= 0
        dl_v = dl_stream[valid]
        c_v = c_of[valid]
        p_v = p_of[valid]
        k_edge = np.minimum((dl_v - b0[c_v]) // 128, kcnt[c_v] - 1)
        e_v = entry_off[c_v] + k_edge
        rel = dl_v - entry_base[e_v]
        assert rel.min() >= 0 and rel.max() < 128
        dl_buf = np.full((total_entries, 128), 999.0, np.float16)
        dl_buf[e_v, p_v] = rel.astype(np.float16)

        idx_buf = np.zeros((128, idx_cols), np.int16)
        for r, g, c0, n in plan:
            seg = idx_stream[c0 * 128: c0 * 128 + n]
            idx_buf[:, c0 * 8: c0 * 8 + n // 16] = _pack_idx_wrap(seg, n)
        per_core.append((idx_buf, np.ascontiguousarray(dl_buf.T)))

    # per-core norm_dst table [128, NSB*4]
    nd_tab = np.zeros((N_CORES, 128, NSB * (SB // 128)), np.float32)
    nd_pad = np.concatenate([norm_dst, np.zeros(N_CORES * OWN + SB, np.float32)])
    for k in range(N_CORES):
        base = k * OWN
        idx = base + np.arange(NSB * SB)
        vals = nd_pad[idx].reshape(NSB * (SB // 128), 128)
        nd_tab[k] = vals.T

    prog = {
        "plan": plan,
        "total_chunks": total_chunks,
        "idx_cols": idx_cols,
        "entry_chunk": entry_chunk,
        "entry_base": entry_base,
        "total_entries": total_entries,
        "group_ranges": group_ranges,
        "range_chunk0": range_chunk0,
        "range_entry0": range_entry0,
        "seg_of_range": seg_of_range,
        "gpad": gpad,
    }
    return prog, per_core, norm_src, nd_tab


def _build_program(prog, with_bias):
    import concourse.bacc as bacc
    import concourse.mybir as mybir
    import concourse.tile as tile
    from concourse.ap import AP

    plan = prog["plan"]
    total_chunks = prog["total_chunks"]
    idx_cols = prog["idx_cols"]
    entry_chunk = prog["entry_chunk"]
    entry_base = prog["entry_base"]
    group_ranges = prog["group_ranges"]
    range_chunk0 = prog["range_chunk0"]
    range_entry0 = prog["range_entry0"]
    seg_of_range = prog["seg_of_range"]
    gpad = prog["gpad"]

    nc = bacc.Bacc(num_swdge_queues=4)
    feat_d = nc.declare_dram_parameter("feat16", [N_NODES, F], mybir.dt.float16, isOutput=False)
    w_d = nc.declare_dram_parameter("w16", [F, F], mybir.dt.float16, isOutput=False)
    bias_d = nc.declare_dram_parameter("biasb", [128, SB], mybir.dt.float32, isOutput=False)
    iota_d = nc.declare_dram_parameter("iota", [128, 128], mybir.dt.float16, isOutput=False)
    idx_d = nc.declare_dram_parameter("idxb", [128, idx_cols], mybir.dt.int16, isOutput=False)
    dl_d = nc.declare_dram_parameter("dlb", [128, prog["total_entries"]], mybir.dt.float16, isOutput=False)
    nd_d = nc.declare_dram_parameter("ndst", [128, NSB * (SB // 128)], mybir.dt.float32, isOutput=False)
    out_d = nc.declare_dram_parameter("out", [NSB * SB, F], mybir.dt.float32, isOutput=True)

    ranges = [(r * RANGE, min((r + 1) * RANGE, N_NODES)) for r in range(NRANGES)]
    max_range_entries = max(range_entry0[r + 1] - range_entry0[r]
                            for r in range(NRANGES))

    with tile.TileContext(nc) as tc:
        with (
            tc.tile_pool(name="const", bufs=1) as constp,
            tc.tile_pool(name="agg", bufs=NSB) as aggp,
            tc.tile_pool(name="et", bufs=10) as etp,
            tc.tile_pool(name="ix", bufs=10) as ixp,
            tc.tile_pool(name="dl", bufs=2) as dlp,
            tc.tile_pool(name="s", bufs=6) as sp,
            tc.tile_pool(name="aggs", bufs=2) as aggsp,
            tc.tile_pool(name="outs", bufs=2) as outsp,
            tc.tile_pool(name="ps", bufs=4, space="PSUM") as psp,
            tc.tile_pool(name="ps2", bufs=2, space="PSUM") as ps2p,
        ):
            w_t = constp.tile([F, F], mybir.dt.float16)
            nc.sync.dma_start(w_t[:], w_d[:])
            bias_t = constp.tile([128, SB], mybir.dt.float32)
            nc.sync.dma_start(bias_t[:], bias_d[:])
            iota_t = constp.tile([128, 128], mybir.dt.float16)
            nc.sync.dma_start(iota_t[:], iota_d[:])
            nd_t = constp.tile([128, NSB * (SB // 128)], mybir.dt.float32)
            nc.sync.dma_start(nd_t[:], nd_d[:])
            zeros_t = constp.tile([128, SB], mybir.dt.float32)
            nc.vector.memset(zeros_t[:], 0.0)

            agg_tiles = [aggp.tile([128, SB], mybir.dt.float32, name=f"agg{s_}", tag="agg")
                         for s_ in range(NSB)]
            init_r = {}
            last_r = {}
            for s in range(NSB):
                rs = [r for r in range(NRANGES) if int(gpad[s, r]) > 0]
                if rs:
                    init_r[s] = rs[0]
                    last_r[s] = rs[-1]

            # calls grouped by (range, segment)
            seg_calls = {}
            for r, g, c0, n in plan:
                seg_calls.setdefault((r, g), []).append((c0, n))

            call_counter = [0]
            # chunk -> (et tile, offset) for the current range
            for r in range(NRANGES):
                lo, hi = ranges[r]
                rc0 = range_chunk0[r]
                re0, re1 = range_entry0[r], range_entry0[r + 1]
                ne_r = re1 - re0
                dlt = dlp.tile([128, max_range_entries], mybir.dt.float16, tag="dl")
                nc.sync.dma_start(dlt[:, :ne_r], dl_d[:, re0:re1])

                chunk_tile = {}
                segs_emitted = 0
                groups = [(s,) + group_ranges[(s, r)] for s in range(NSB)
                          if group_ranges[(s, r)][1] > group_ranges[(s, r)][0]]
                gi = 0
                for g in range(seg_of_range[r]):
                    calls = seg_calls[(r, g)]
                    seg_c0 = calls[0][0]
                    seg_nch = sum(n for _, n in calls) // 128
                    et = etp.tile([128, seg_nch * F], mybir.dt.float16, tag="et")
                    ix = ixp.tile([128, (SEG_ROWS // 128) * 8], mybir.dt.int16, tag="ix")
                    nc.sync.dma_start(ix[:, : seg_nch * 8],
                                      idx_d[:, seg_c0 * 8: (seg_c0 + seg_nch) * 8])
                    for c0, n in calls:
                        rel = c0 - seg_c0
                        nc.gpsimd.dma_gather(
                            out_ap=et[:, rel * F: (rel + n // 128) * F].rearrange(
                                "p (c e) -> p c e", e=F),
                            in_ap=feat_d[lo:hi, :],
                            idxs_ap=ix[:, rel * 8: rel * 8 + n // 16],
                            num_idxs=n,
                            num_idxs_reg=n,
                            elem_size=F,
                            queue_num=call_counter[0] % 4,
                        )
                        call_counter[0] += 1
                    for c in range(seg_c0, seg_c0 + seg_nch):
                        chunk_tile[c] = (et, c - seg_c0)
                    segs_emitted += 1

                    # emit compute for groups whose last chunk is now gathered
                    last_ready = seg_c0 + seg_nch - 1
                    while gi < len(groups) and groups[gi][2] - 1 <= last_ready:
                        s, gc0, gc1, ge0, ge1 = groups[gi]
                        gi += 1
                        psT = psp.tile([128, SB], mybir.dt.float32, space="PSUM")
                        nc.scalar.copy(psT[:], zeros_t[:])
                        nent = ge1 - ge0
                        for p0 in range(0, nent, PIECE):
                            pe = min(p0 + PIECE, nent)
                            np_ = pe - p0
                            st = sp.tile([128, PIECE * 128], mybir.dt.float16, tag="s")
                            bsl = dlt[:, ge0 - re0 + p0: ge0 - re0 + pe]
                            b = bsl.to_broadcast([128, np_, 128])
                            a = iota_t[:]
                            i2 = AP(a.tensor, a.offset,
                                    [list(a.ap[0]), [0, np_], list(a.ap[1])])
                            nc.vector.tensor_tensor(
                                out=st[:, : np_ * 128].rearrange(
                                    "p (c j) -> p c j", j=128),
                                in0=b, in1=i2, op=mybir.AluOpType.is_equal,
                            )
                            for kk in range(p0, pe):
                                c = int(entry_chunk[ge0 + kk])
                                base = int(entry_base[ge0 + kk])
                                ett, off = chunk_tile[c]
                                nc.tensor.matmul(
                                    out=psT[:, base: base + 128],
                                    lhsT=ett[:, off * F: (off + 1) * F],
                                    rhs=st[:, (kk - p0) * 128: (kk - p0 + 1) * 128],
                                    start=False,
                                    stop=(kk == nent - 1),
                                )
                        if r == init_r[s]:
                            nc.scalar.copy(agg_tiles[s][:], psT[:])
                        else:
                            nc.vector.tensor_tensor(
                                out=agg_tiles[s][:], in0=agg_tiles[s][:],
                                in1=psT[:], op=mybir.AluOpType.add,
                            )
                        if r == last_r[s]:
                            aggT = aggsp.tile([128, SB], mybir.dt.float16)
                            nc.scalar.copy(aggT[:], agg_tiles[s][:])
                            ps2 = ps2p.tile([128, SB], mybir.dt.float32, space="PSUM")
                            for j in range(SB // F):
                                nc.tensor.matmul(
                                    out=ps2[:, j * F: (j + 1) * F],
                                    lhsT=aggT[:, j * F: (j + 1) * F],
                                    rhs=w_t[:],
                                    start=True,
                                    stop=True,
                                )
                            ot = outsp.tile([128, SB], mybir.dt.float32)
                            for j in range(SB // F):
                                nc.scalar.activation(
                                    ot[:, j * F: (j + 1) * F],
                                    ps2[:, j * F: (j + 1) * F],
                                    mybir.ActivationFunctionType.Copy,
                                    scale=nd_t[:, s * (SB // F) + j:
                                               s * (SB // F) + j + 1],
                                )
                            if with_bias:
                                nc.gpsimd.tensor_tensor(
                                    out=ot[:], in0=ot[:], in1=bias_t[:],
                                    op=mybir.AluOpType.add,
                                )
                            nc.sync.dma_start(
                                out_d[s * SB: (s + 1) * SB, :].rearrange(
                                    "(j p) f -> p j f", p=128),
                                ot[:].rearrange("p (j f) -> p j f", f=F),
                            )
    nc.finalize()
    return nc


def kernel(feat, weight, bias, src, dst):
    _install_walrus_passes()
    from concourse.bass_utils import run_bass_kernel_spmd

    feat = np.asarray(feat, dtype=np.float32)
    weight = np.asarray(weight, dtype=np.float32)
    bias = np.asarray(bias, dtype=np.float32)

    prog, per_core, norm_src, nd_tab = _preprocess(src, dst)
    feat16 = np.ascontiguousarray((feat * norm_src[:, None]).astype(np.float16))
    w16 = np.ascontiguousarray(weight.astype(np.float16))
    with_bias = bool(np.any(bias != 0.0))
    nc = _build_program(prog, with_bias)

    bias_b = np.broadcast_to(np.tile(bias, SB // F)[None, :], (128, SB)).copy()
    iota = np.broadcast_to(np.arange(128, dtype=np.float16)[None, :], (128, 128)).copy()

    in_maps = []
    for k in range(N_CORES):
        idx_buf, dl_buf = per_core[k]
        in_maps.append({
            "feat16": feat16,
            "w16": w16,
            "biasb": bias_b,
            "iota": iota,
            "idxb": idx_buf,
            "dlb": dl_buf,
            "ndst": np.ascontiguousarray(nd_tab[k]),
        })
    res = run_bass_kernel_spmd(nc, in_maps, list(range(N_CORES)))
    out = np.empty((N_CORES * OWN, F), np.float32)
    for k in range(N_CORES):
        out[k * OWN: (k + 1) * OWN] = res.results[k]["out"][:OWN]
    return out[:N_NODES]


# revision 14
# speedup vs baseline: 1.0067x; 1.0067x over previous
"""GCN layer v5 on 8 Trainium2 NeuronCores.

v3 + range-major gather packing:
- Edges ordered (range, sb, dloc) so gather calls span sb boundaries within
  a range (same in_ap): ~240 nearly-full 896-row calls instead of 319
  (the SWDGE ring caps calls at 64 descs = 896 rows, and per-call fixed
  cost ~1.1us on the Pool engine is the kernel's bottleneck).
- Gathers land in fixed 3584-row et segment tiles (4 calls each).
- psT [128,512] PSUM partials per (sb, range) group are accumulated into
  25 persistent SBUF agg tiles (Act zero-inits psT, DVE adds partials);
  after the last range sweep each sb runs the W-matmul + norm_dst output.
- One-hot S via broadcast tensor_tensor (is_equal) per group, fp16;
  feat pre-scaled by norm_src on host; norm_dst in Act output copies.
"""
import numpy as np

N_NODES = 100000
N_EDGES = 1600000
F = 128
N_CORES = 8
OWN = 12544
SB = 512
NSB = 25
RANGE = 32768
NRANGES = 4
MAX_CALL = 896      # dma_gather single-call cap (ring = 64 descs incl sem)
SEG_CALLS = 4       # gather calls per et segment tile
SEG_ROWS = MAX_CALL * SEG_CALLS   # 3584 rows = 28 chunks
PIECE = 24          # one-hot entries per DVE instruction


def _install_walrus_passes():
    import concourse.bass_utils as bu

    def patched(tmpdir, inp="bir.json", outp="file.neff", arch=None, *, dve_root=None):
        from pathlib import Path
        cmd = [
            bu.get_walrus_driver(),
            "--pass",
            "birverifier,dynamic_dma_scan,runtime_memory_reservation,"
            "dynamic_dma_setup,lower_act,lower_dve,lower_ap_offset,"
            "codegen,neff_packager",
            "-i", inp,
            "--neff-output-filename", outp,
            "--enable-birsim=true",
            "--mem-mode=physical",
            "--policy=0",
            "--enable-ldw-opt=false",
            "--assign-static-dmas-to-sp=false",
            "--dram-page-size=256",
            "--enable-neff-debug-info=true",
            "--jobs", "8",
            "--dynamic-dma-scratch-size-per-partition=16384",
            *bu.get_walrus_args(
                bu.get_bir_arch(tmpdir, inp) if arch is None else arch,
                tmpdir, dve_root=dve_root,
            ),
        ]
        result = bu.run_command(cmd, cwd=tmpdir)
        if result is not None:
            (Path(tmpdir) / "log.txt").write_text(result.stdout)
        return f"{tmpdir}/{outp}"

    bu.bir_verify_and_optimise = patched


def _pack_idx_wrap(idx_i16: np.ndarray, cap: int) -> np.ndarray:
    w = np.zeros((16, cap // 16), np.int16)
    j = np.arange(len(idx_i16))
    w[j % 16, j // 16] = idx_i16
    return np.tile(w, (8, 1))


def _preprocess(src: np.ndarray, dst: np.ndarray):
    """Returns (prog, per_core, norm_src, norm_dst_percore)."""
    src = np.asarray(src).astype(np.int64)
    dst = np.asarray(dst).astype(np.int64)

    out_deg = np.bincount(src, minlength=N_NODES).astype(np.float32)
    in_deg = np.bincount(dst, minlength=N_NODES).astype(np.float32)
    norm_src = 1.0 / np.sqrt(np.clip(out_deg, 1.0, None))
    norm_dst = 1.0 / np.sqrt(np.clip(in_deg, 1.0, None))

    core = np.minimum(dst // OWN, N_CORES - 1)
    dst_local = dst - core * OWN
    sb = dst_local // SB
    dloc = dst_local - sb * SB
    rng = src // RANGE

    sizes = np.zeros((N_CORES, NSB, NRANGES), np.int64)
    np.add.at(sizes, (core, sb, rng), 1)
    gmax = sizes.max(axis=0)
    gpad = ((gmax + 127) // 128) * 128          # [NSB, NRANGES]

    # range-major chunk grid: chunks of groups (s, r) laid out r-major
    group_chunk0 = {}
    total_chunks = 0
    range_chunk0 = []
    for r in range(NRANGES):
        range_chunk0.append(total_chunks)
        for s in range(NSB):
            group_chunk0[(s, r)] = total_chunks
            total_chunks += int(gpad[s, r]) // 128
    range_chunk0.append(total_chunks)

    # gather call plan: per range, calls of <=896 rows inside 3584-row
    # (28-chunk) segments; (r, seg_idx, chunk_offset, n_idx)
    plan = []
    seg_of_range = []      # per range: number of segments
    for r in range(NRANGES):
        c0 = range_chunk0[r]
        c1 = range_chunk0[r + 1]
        nch_r = c1 - c0
        nseg = (nch_r + SEG_ROWS // 128 - 1) // (SEG_ROWS // 128)
        seg_of_range.append(nseg)
        for g in range(nseg):
            sc0 = c0 + g * (SEG_ROWS // 128)
            sc1 = min(sc0 + SEG_ROWS // 128, c1)
            rows = (sc1 - sc0) * 128
            off = 0
            while off < rows:
                take = min(MAX_CALL, rows - off)
                plan.append((r, g, sc0 + off // 128, take))
                off += take
    idx_cols = total_chunks * 8

    # per-core padded streams in range-major order
    dlmin = np.full(total_chunks, SB, np.int64)
    dlmax = np.full(total_chunks, -1, np.int64)
    core_streams = []
    for k in range(N_CORES):
        m = core == k
        e_sb, e_rng = sb[m], rng[m]
        e_src, e_dl = src[m], dloc[m]
        order = np.lexsort((e_dl, e_sb, e_rng))
        e_sb, e_rng = e_sb[order], e_rng[order]
        e_src, e_dl = e_src[order], e_dl[order]

        idx_stream = np.zeros(total_chunks * 128, np.int16)
        dl_stream = np.full(total_chunks * 128, -1, np.int64)  # -1 = pad

        gsizes = np.zeros((NSB, NRANGES), np.int64)
        np.add.at(gsizes, (e_sb, e_rng), 1)
        acc = 0
        for r in range(NRANGES):
            for s in range(NSB):
                n_real = int(gsizes[s, r])
                if n_real == 0:
                    continue
                p0 = group_chunk0[(s, r)] * 128
                sl = slice(p0, p0 + n_real)
                idx_stream[sl] = (e_src[acc:acc + n_real] - r * RANGE).astype(np.int16)
                dl_stream[sl] = e_dl[acc:acc + n_real]
                acc += n_real
        core_streams.append((idx_stream, dl_stream))

        dl2 = dl_stream.reshape(total_chunks, 128)
        v2 = dl2 >= 0
        has = v2.any(axis=1)
        cmin = np.where(has, np.where(v2, dl2, SB).min(axis=1), SB)
        cmax = np.where(has, np.where(v2, dl2, -1).max(axis=1), -1)
        dlmin = np.minimum(dlmin, cmin)
        dlmax = np.maximum(dlmax, cmax)

    # entries: per chunk, ceil(span/128) windows starting at min(dlmin, 384)
    has_any = dlmax >= 0
    b0 = np.minimum(np.where(has_any, dlmin, 0), SB - 128)
    kcnt = np.where(has_any, (dlmax - b0) // 128 + 1, 1)
    entry_chunk = []
    entry_base = []
    entry_off = np.zeros(total_chunks + 1, np.int64)
    for c in range(total_chunks):
        entry_off[c] = len(entry_chunk)
        for k in range(int(kcnt[c])):
            entry_chunk.append(c)
            entry_base.append(min(int(b0[c]) + 128 * k, SB - 128))
        entry_off[total_chunks] = len(entry_chunk)
    entry_chunk = np.array(entry_chunk, np.int64)
    entry_base = np.array(entry_base, np.int64)
    total_entries = len(entry_chunk)

    # per-group chunk/entry ranges
    group_ranges = {}      # (s, r) -> (gc0, gc1, ge0, ge1)
    for r in range(NRANGES):
        for s in range(NSB):
            gc0 = group_chunk0[(s, r)]
            gc1 = gc0 + int(gpad[s, r]) // 128
            group_ranges[(s, r)] = (gc0, gc1,
                                    int(entry_off[gc0]), int(entry_off[gc1]))
    range_entry0 = [int(entry_off[range_chunk0[r]]) for r in range(NRANGES)]
    range_entry0.append(total_entries)

    # per-core dl buffers [128, total_entries] fp16 (999 = no match)
    per_core = []
    for k in range(N_CORES):
        idx_stream, dl_stream = core_streams[k]
        pos = np.arange(total_chunks * 128)
        c_of = pos // 128
        p_of = pos % 128
        valid = dl_stream >= 0
        dl_v = dl_stream[valid]
        c_v = c_of[valid]
        p_v = p_of[valid]
        k_edge = np.minimum((dl_v - b0[c_v]) // 128, kcnt[c_v] - 1)
        e_v = entry_off[c_v] + k_edge
        rel = dl_v - entry_base[e_v]
        assert rel.min() >= 0 and rel.max() < 128
        dl_buf = np.full((total_entries, 128), 999.0, np.float16)
        dl_buf[e_v, p_v] = rel.astype(np.float16)

        idx_buf = np.zeros((128, idx_cols), np.int16)
        for r, g, c0, n in plan:
            seg = idx_stream[c0 * 128: c0 * 128 + n]
            idx_buf[:, c0 * 8: c0 * 8 + n // 16] = _pack_idx_wrap(seg, n)
        per_core.append((idx_buf, np.ascontiguousarray(dl_buf.T)))

    # per-core norm_dst table [128, NSB*4]
    nd_tab = np.zeros((N_CORES, 128, NSB * (SB // 128)), np.float32)
    nd_pad = np.concatenate([norm_dst, np.zeros(N_CORES * OWN + SB, np.float32)])
    for k in range(N_CORES):
        base = k * OWN
        idx = base + np.arange(NSB * SB)
        vals = nd_pad[idx].reshape(NSB * (SB // 128), 128)
        nd_tab[k] = vals.T

    prog = {
        "plan": plan,
        "total_chunks": total_chunks,
        "idx_cols": idx_cols,
        "entry_chunk": entry_chunk,
        "entry_base": entry_base,
        "total_entries": total_entries,
        "group_ranges": group_ranges,
        "range_chunk0": range_chunk0,
        "range_entry0": range_entry0,
        "seg_of_range": seg_of_range,
        "gpad": gpad,
    }
    return prog, per_core, norm_src, nd_tab


def _build_program(prog, with_bias):
    import concourse.bacc as bacc
    import concourse.mybir as mybir
    import concourse.tile as tile
    from concourse.ap import AP

    plan = prog["plan"]
    total_chunks = prog["total_chunks"]
    idx_cols = prog["idx_cols"]
    entry_chunk = prog["entry_chunk"]
    entry_base = prog["entry_base"]
    group_ranges = prog["group_ranges"]
    range_chunk0 = prog["range_chunk0"]
    range_entry0 = prog["range_entry0"]
    seg_of_range = prog["seg_of_range"]
    gpad = prog["gpad"]

    nc = bacc.Bacc(num_swdge_queues=4)
    feat_d = nc.declare_dram_parameter("feat16", [N_NODES, F], mybir.dt.float16, isOutput=False)
    w_d = nc.declare_dram_parameter("w16", [F, F], mybir.dt.float16, isOutput=False)
    bias_d = nc.declare_dram_parameter("biasb", [128, SB], mybir.dt.float32, isOutput=False)
    iota_d = nc.declare_dram_parameter("iota", [128, 128], mybir.dt.float16, isOutput=False)
    idx_d = nc.declare_dram_parameter("idxb", [128, idx_cols], mybir.dt.int16, isOutput=False)
    dl_d = nc.declare_dram_parameter("dlb", [128, prog["total_entries"]], mybir.dt.float16, isOutput=False)
    nd_d = nc.declare_dram_parameter("ndst", [128, NSB * (SB // 128)], mybir.dt.float32, isOutput=False)
    out_d = nc.declare_dram_parameter("out", [NSB * SB, F], mybir.dt.float32, isOutput=True)

    ranges = [(r * RANGE, min((r + 1) * RANGE, N_NODES)) for r in range(NRANGES)]
    max_range_entries = max(range_entry0[r + 1] - range_entry0[r]
                            for r in range(NRANGES))
    max_range_chunks = max(range_chunk0[r + 1] - range_chunk0[r]
                           for r in range(NRANGES))

    with tile.TileContext(nc) as tc:
        with (
            tc.tile_pool(name="const", bufs=1) as constp,
            tc.tile_pool(name="agg", bufs=NSB) as aggp,
            tc.tile_pool(name="et", bufs=4) as etp,
            tc.tile_pool(name="ix", bufs=2) as ixp,
            tc.tile_pool(name="dl", bufs=2) as dlp,
            tc.tile_pool(name="s", bufs=4) as sp,
            tc.tile_pool(name="aggs", bufs=2) as aggsp,
            tc.tile_pool(name="outs", bufs=2) as outsp,
            tc.tile_pool(name="ps", bufs=2, space="PSUM") as psp,
            tc.tile_pool(name="ps2", bufs=2, space="PSUM") as ps2p,
        ):
            w_t = constp.tile([F, F], mybir.dt.float16)
            nc.sync.dma_start(w_t[:], w_d[:])
            bias_t = constp.tile([128, SB], mybir.dt.float32)
            nc.sync.dma_start(bias_t[:], bias_d[:])
            iota_t = constp.tile([128, 128], mybir.dt.float16)
            nc.sync.dma_start(iota_t[:], iota_d[:])
            nd_t = constp.tile([128, NSB * (SB // 128)], mybir.dt.float32)
            nc.sync.dma_start(nd_t[:], nd_d[:])
            zeros_t = constp.tile([128, SB], mybir.dt.float32)
            nc.vector.memset(zeros_t[:], 0.0)

            agg_tiles = [aggp.tile([128, SB], mybir.dt.float32, name=f"agg{s_}", tag="agg")
                         for s_ in range(NSB)]
            init_r = {}
            last_r = {}
            for s in range(NSB):
                rs = [r for r in range(NRANGES) if int(gpad[s, r]) > 0]
                if rs:
                    init_r[s] = rs[0]
                    last_r[s] = rs[-1]

            # calls grouped by (range, segment)
            seg_calls = {}
            for r, g, c0, n in plan:
                seg_calls.setdefault((r, g), []).append((c0, n))

            call_counter = [0]
            # chunk -> (et tile, offset) for the current range
            for r in range(NRANGES):
                lo, hi = ranges[r]
                rc0 = range_chunk0[r]
                re0, re1 = range_entry0[r], range_entry0[r + 1]
                ne_r = re1 - re0
                dlt = dlp.tile([128, max_range_entries], mybir.dt.float16, tag="dl")
                nc.sync.dma_start(dlt[:, :ne_r], dl_d[:, re0:re1])
                nch_r = range_chunk0[r + 1] - rc0
                ixr = ixp.tile([128, max_range_chunks * 8], mybir.dt.int16, tag="ix")
                nc.sync.dma_start(ixr[:, : nch_r * 8],
                                  idx_d[:, rc0 * 8: (rc0 + nch_r) * 8])

                chunk_tile = {}
                segs_emitted = 0
                groups = [(s,) + group_ranges[(s, r)] for s in range(NSB)
                          if group_ranges[(s, r)][1] > group_ranges[(s, r)][0]]
                gi = 0
                for g in range(seg_of_range[r]):
                    calls = seg_calls[(r, g)]
                    seg_c0 = calls[0][0]
                    seg_nch = sum(n for _, n in calls) // 128
                    et = etp.tile([128, seg_nch * F], mybir.dt.float16, tag="et")
                    for c0, n in calls:
                        rel = c0 - seg_c0
                        nc.gpsimd.dma_gather(
                            out_ap=et[:, rel * F: (rel + n // 128) * F].rearrange(
                                "p (c e) -> p c e", e=F),
                            in_ap=feat_d[lo:hi, :],
                            idxs_ap=ixr[:, (c0 - rc0) * 8: (c0 - rc0) * 8 + n // 16],
                            num_idxs=n,
                            num_idxs_reg=n,
                            elem_size=F,
                            queue_num=call_counter[0] % 4,
                        )
                        call_counter[0] += 1
                    for c in range(seg_c0, seg_c0 + seg_nch):
                        chunk_tile[c] = (et, c - seg_c0)
                    segs_emitted += 1

                    # emit compute for groups whose last chunk is now gathered
                    last_ready = seg_c0 + seg_nch - 1
                    while gi < len(groups) and groups[gi][2] - 1 <= last_ready:
                        s, gc0, gc1, ge0, ge1 = groups[gi]
                        gi += 1
                        psT = psp.tile([128, SB], mybir.dt.float32, space="PSUM")
                        nc.scalar.copy(psT[:], zeros_t[:])
                        nent = ge1 - ge0
                        for p0 in range(0, nent, PIECE):
                            pe = min(p0 + PIECE, nent)
                            np_ = pe - p0
                            st = sp.tile([128, PIECE * 128], mybir.dt.float16, tag="s")
                            bsl = dlt[:, ge0 - re0 + p0: ge0 - re0 + pe]
                            b = bsl.to_broadcast([128, np_, 128])
                            a = iota_t[:]
                            i2 = AP(a.tensor, a.offset,
                                    [list(a.ap[0]), [0, np_], list(a.ap[1])])
                            nc.vector.tensor_tensor(
                                out=st[:, : np_ * 128].rearrange(
                                    "p (c j) -> p c j", j=128),
                                in0=b, in1=i2, op=mybir.AluOpType.is_equal,
                            )
                            for kk in range(p0, pe):
                                c = int(entry_chunk[ge0 + kk])
                                base = int(entry_base[ge0 + kk])
                                ett, off = chunk_tile[c]
                                nc.tensor.matmul(
                                    out=psT[:, base: base + 128],
                                    lhsT=ett[:, off * F: (off + 1) * F],
                                    rhs=st[:, (kk - p0) * 128: (kk - p0 + 1) * 128],
                                    start=False,
                                    stop=(kk == nent - 1),
                                )
                        if r == init_r[s]:
                            nc.scalar.copy(agg_tiles[s][:], psT[:])
                        else:
                            nc.vector.tensor_tensor(
                                out=agg_tiles[s][:], in0=agg_tiles[s][:],
                                in1=psT[:], op=mybir.AluOpType.add,
                            )
                        if r == last_r[s]:
                            aggT = aggsp.tile([128, SB], mybir.dt.float16)
                            nc.scalar.copy(aggT[:], agg_tiles[s][:])
                            ps2 = ps2p.tile([128, SB], mybir.dt.float32, space="PSUM")
                            for j in range(SB // F):
                                nc.tensor.matmul(
                                    out=ps2[:, j * F: (j + 1) * F],
                                    lhsT=aggT[:, j * F: (j + 1) * F],
                                    rhs=w_t[:],
                                    start=True,
                                    stop=True,
                                )
                            ot = outsp.tile([128, SB], mybir.dt.float32)
                            for j in range(SB // F):
                                nc.scalar.activation(
                                    ot[:, j * F: (j + 1) * F],
                                    ps2[:, j * F: (j + 1) * F],
                                    mybir.ActivationFunctionType.Copy,
                                    scale=nd_t[:, s * (SB // F) + j:
                                               s * (SB // F) + j + 1],
                                )
                            if with_bias:
                                nc.gpsimd.tensor_tensor(
                                    out=ot[:], in0=ot[:], in1=bias_t[:],
                                    op=mybir.AluOpType.add,
                                )
                            nc.sync.dma_start(
                                out_d[s * SB: (s + 1) * SB, :].rearrange(
                                    "(j p) f -> p j f", p=128),
                                ot[:].rearrange("p (j f) -> p j f", f=F),
                            )
    nc.finalize()
    return nc


def kernel(feat, weight, bias, src, dst):
    _install_walrus_passes()
    from concourse.bass_utils import run_bass_kernel_spmd

    feat = np.asarray(feat, dtype=np.float32)
    weight = np.asarray(weight, dtype=np.float32)
    bias = np.asarray(bias, dtype=np.float32)

    prog, per_core, norm_src, nd_tab = _preprocess(src, dst)
    feat16 = np.ascontiguousarray((feat * norm_src[:, None]).astype(np.float16))
    w16 = np.ascontiguousarray(weight.astype(np.float16))
    with_bias = bool(np.any(bias != 0.0))
    nc = _build_program(prog, with_bias)

    bias_b = np.broadcast_to(np.tile(bias, SB // F)[None, :], (128, SB)).copy()
    iota = np.broadcast_to(np.arange(128, dtype=np.float16)[None, :], (128, 128)).copy()

    in_maps = []
    for k in range(N_CORES):
        idx_buf, dl_buf = per_core[k]
        in_maps.append({
            "feat16": feat16,
            "w16": w16,
            "biasb": bias_b,
            "iota": iota,
            "idxb": idx_buf,
            "dlb": dl_buf,
            "ndst": np.ascontiguousarray(nd_tab[k]),
        })
    res = run_bass_kernel_spmd(nc, in_maps, list(range(N_CORES)))
    out = np.empty((N_CORES * OWN, F), np.float32)
    for k in range(N_CORES):
        out[k * OWN: (k + 1) * OWN] = res.results[k]["out"][:OWN]
    return out[:N_NODES]


# revision 16
# speedup vs baseline: 1.0606x; 1.0535x over previous
"""GCN layer v5 on 8 Trainium2 NeuronCores.

v3 + range-major gather packing:
- Edges ordered (range, sb, dloc) so gather calls span sb boundaries within
  a range (same in_ap): ~240 nearly-full 896-row calls instead of 319
  (the SWDGE ring caps calls at 64 descs = 896 rows, and per-call fixed
  cost ~1.1us on the Pool engine is the kernel's bottleneck).
- Gathers land in fixed 3584-row et segment tiles (4 calls each).
- psT [128,512] PSUM partials per (sb, range) group are accumulated into
  25 persistent SBUF agg tiles (Act zero-inits psT, DVE adds partials);
  after the last range sweep each sb runs the W-matmul + norm_dst output.
- One-hot S via broadcast tensor_tensor (is_equal) per group, fp16;
  feat pre-scaled by norm_src on host; norm_dst in Act output copies.
"""
import numpy as np

N_NODES = 100000
N_EDGES = 1600000
F = 128
N_CORES = 8
OWN = 12544
SB = 512
NSB = 25
RANGE = 32768
NRANGES = 4
MAX_CALL = 896      # dma_gather single-call cap (ring = 64 descs incl sem)
SEG_CALLS = 4       # gather calls per et segment tile
SEG_ROWS = MAX_CALL * SEG_CALLS   # 3584 rows = 28 chunks
PIECE = 24          # one-hot entries per DVE instruction
# process the tiny last src-range FIRST so the 25 per-sb output chains
# (fired during the final sweep) overlap a full-size range's gathers
RANGE_ORDER = (3, 0, 1, 2)


def _install_walrus_passes():
    import concourse.bass_utils as bu

    def patched(tmpdir, inp="bir.json", outp="file.neff", arch=None, *, dve_root=None):
        from pathlib import Path
        cmd = [
            bu.get_walrus_driver(),
            "--pass",
            "birverifier,dynamic_dma_scan,runtime_memory_reservation,"
            "dynamic_dma_setup,lower_act,lower_dve,lower_ap_offset,"
            "codegen,neff_packager",
            "-i", inp,
            "--neff-output-filename", outp,
            "--enable-birsim=true",
            "--mem-mode=physical",
            "--policy=0",
            "--enable-ldw-opt=false",
            "--assign-static-dmas-to-sp=false",
            "--dram-page-size=256",
            "--enable-neff-debug-info=true",
            "--jobs", "8",
            "--dynamic-dma-scratch-size-per-partition=16384",
            *bu.get_walrus_args(
                bu.get_bir_arch(tmpdir, inp) if arch is None else arch,
                tmpdir, dve_root=dve_root,
            ),
        ]
        result = bu.run_command(cmd, cwd=tmpdir)
        if result is not None:
            (Path(tmpdir) / "log.txt").write_text(result.stdout)
        return f"{tmpdir}/{outp}"

    bu.bir_verify_and_optimise = patched


def _pack_idx_wrap(idx_i16: np.ndarray, cap: int) -> np.ndarray:
    w = np.zeros((16, cap // 16), np.int16)
    j = np.arange(len(idx_i16))
    w[j % 16, j // 16] = idx_i16
    return np.tile(w, (8, 1))


def _preprocess(src: np.ndarray, dst: np.ndarray):
    """Returns (prog, per_core, norm_src, norm_dst_percore)."""
    src = np.asarray(src).astype(np.int64)
    dst = np.asarray(dst).astype(np.int64)

    out_deg = np.bincount(src, minlength=N_NODES).astype(np.float32)
    in_deg = np.bincount(dst, minlength=N_NODES).astype(np.float32)
    norm_src = 1.0 / np.sqrt(np.clip(out_deg, 1.0, None))
    norm_dst = 1.0 / np.sqrt(np.clip(in_deg, 1.0, None))

    core = np.minimum(dst // OWN, N_CORES - 1)
    dst_local = dst - core * OWN
    sb = dst_local // SB
    dloc = dst_local - sb * SB
    rng = src // RANGE

    sizes = np.zeros((N_CORES, NSB, NRANGES), np.int64)
    np.add.at(sizes, (core, sb, rng), 1)
    gmax = sizes.max(axis=0)
    gpad = ((gmax + 127) // 128) * 128          # [NSB, NRANGES]

    # range-major chunk grid: chunks of groups (s, r) laid out r-major
    group_chunk0 = {}
    total_chunks = 0
    range_chunk0 = []
    for r in RANGE_ORDER:
        range_chunk0.append(total_chunks)
        for s in range(NSB):
            group_chunk0[(s, r)] = total_chunks
            total_chunks += int(gpad[s, r]) // 128
    range_chunk0.append(total_chunks)

    # gather call plan: per range, calls of <=896 rows inside 3584-row
    # (28-chunk) segments; (r, seg_idx, chunk_offset, n_idx)
    plan = []
    seg_of_range = []      # per range position: number of segments
    for p_ in range(NRANGES):
        r = RANGE_ORDER[p_]
        c0 = range_chunk0[p_]
        c1 = range_chunk0[p_ + 1]
        nch_r = c1 - c0
        nseg = (nch_r + SEG_ROWS // 128 - 1) // (SEG_ROWS // 128)
        seg_of_range.append(nseg)
        for g in range(nseg):
            sc0 = c0 + g * (SEG_ROWS // 128)
            sc1 = min(sc0 + SEG_ROWS // 128, c1)
            rows = (sc1 - sc0) * 128
            off = 0
            while off < rows:
                take = min(MAX_CALL, rows - off)
                plan.append((r, g, sc0 + off // 128, take))
                off += take
    idx_cols = total_chunks * 8

    # per-core padded streams in range-major order
    dlmin = np.full(total_chunks, SB, np.int64)
    dlmax = np.full(total_chunks, -1, np.int64)
    core_streams = []
    for k in range(N_CORES):
        m = core == k
        e_sb, e_rng = sb[m], rng[m]
        e_src, e_dl = src[m], dloc[m]
        pos_map = np.zeros(NRANGES, np.int64)
        pos_map[list(RANGE_ORDER)] = np.arange(NRANGES)
        order = np.lexsort((e_dl, e_sb, pos_map[e_rng]))
        e_sb, e_rng = e_sb[order], e_rng[order]
        e_src, e_dl = e_src[order], e_dl[order]

        idx_stream = np.zeros(total_chunks * 128, np.int16)
        dl_stream = np.full(total_chunks * 128, -1, np.int64)  # -1 = pad

        gsizes = np.zeros((NSB, NRANGES), np.int64)
        np.add.at(gsizes, (e_sb, e_rng), 1)
        acc = 0
        for r in RANGE_ORDER:
            for s in range(NSB):
                n_real = int(gsizes[s, r])
                if n_real == 0:
                    continue
                p0 = group_chunk0[(s, r)] * 128
                sl = slice(p0, p0 + n_real)
                idx_stream[sl] = (e_src[acc:acc + n_real] - r * RANGE).astype(np.int16)
                dl_stream[sl] = e_dl[acc:acc + n_real]
                acc += n_real
        core_streams.append((idx_stream, dl_stream))

        dl2 = dl_stream.reshape(total_chunks, 128)
        v2 = dl2 >= 0
        has = v2.any(axis=1)
        cmin = np.where(has, np.where(v2, dl2, SB).min(axis=1), SB)
        cmax = np.where(has, np.where(v2, dl2, -1).max(axis=1), -1)
        dlmin = np.minimum(dlmin, cmin)
        dlmax = np.maximum(dlmax, cmax)

    # entries: per chunk, ceil(span/128) windows starting at min(dlmin, 384)
    has_any = dlmax >= 0
    b0 = np.minimum(np.where(has_any, dlmin, 0), SB - 128)
    kcnt = np.where(has_any, (dlmax - b0) // 128 + 1, 1)
    entry_chunk = []
    entry_base = []
    entry_off = np.zeros(total_chunks + 1, np.int64)
    for c in range(total_chunks):
        entry_off[c] = len(entry_chunk)
        for k in range(int(kcnt[c])):
            entry_chunk.append(c)
            entry_base.append(min(int(b0[c]) + 128 * k, SB - 128))
        entry_off[total_chunks] = len(entry_chunk)
    entry_chunk = np.array(entry_chunk, np.int64)
    entry_base = np.array(entry_base, np.int64)
    total_entries = len(entry_chunk)

    # per-group chunk/entry ranges
    group_ranges = {}      # (s, r) -> (gc0, gc1, ge0, ge1)
    for r in range(NRANGES):
        for s in range(NSB):
            gc0 = group_chunk0[(s, r)]
            gc1 = gc0 + int(gpad[s, r]) // 128
            group_ranges[(s, r)] = (gc0, gc1,
                                    int(entry_off[gc0]), int(entry_off[gc1]))
    range_entry0 = [int(entry_off[range_chunk0[r]]) for r in range(NRANGES)]
    range_entry0.append(total_entries)

    # per-core dl buffers [128, total_entries] fp16 (999 = no match)
    per_core = []
    for k in range(N_CORES):
        idx_stream, dl_stream = core_streams[k]
        pos = np.arange(total_chunks * 128)
        c_of = pos // 128
        p_of = pos % 128
        valid = dl_stream >= 0
        dl_v = dl_stream[valid]
        c_v = c_of[valid]
        p_v = p_of[valid]
        k_edge = np.minimum((dl_v - b0[c_v]) // 128, kcnt[c_v] - 1)
        e_v = entry_off[c_v] + k_edge
        rel = dl_v - entry_base[e_v]
        assert rel.min() >= 0 and rel.max() < 128
        dl_buf = np.full((total_entries, 128), 999.0, np.float16)
        dl_buf[e_v, p_v] = rel.astype(np.float16)

        idx_buf = np.zeros((128, idx_cols), np.int16)
        for r, g, c0, n in plan:
            seg = idx_stream[c0 * 128: c0 * 128 + n]
            idx_buf[:, c0 * 8: c0 * 8 + n // 16] = _pack_idx_wrap(seg, n)
        per_core.append((idx_buf, np.ascontiguousarray(dl_buf.T)))

    # per-core norm_dst table [128, NSB*4]
    nd_tab = np.zeros((N_CORES, 128, NSB * (SB // 128)), np.float32)
    nd_pad = np.concatenate([norm_dst, np.zeros(N_CORES * OWN + SB, np.float32)])
    for k in range(N_CORES):
        base = k * OWN
        idx = base + np.arange(NSB * SB)
        vals = nd_pad[idx].reshape(NSB * (SB // 128), 128)
        nd_tab[k] = vals.T

    prog = {
        "plan": plan,
        "total_chunks": total_chunks,
        "idx_cols": idx_cols,
        "entry_chunk": entry_chunk,
        "entry_base": entry_base,
        "total_entries": total_entries,
        "group_ranges": group_ranges,
        "range_chunk0": range_chunk0,
        "range_entry0": range_entry0,
        "seg_of_range": seg_of_range,
        "gpad": gpad,
    }
    return prog, per_core, norm_src, nd_tab


def _build_program(prog, with_bias):
    import concourse.bacc as bacc
    import concourse.mybir as mybir
    import concourse.tile as tile
    from concourse.ap import AP

    plan = prog["plan"]
    total_chunks = prog["total_chunks"]
    idx_cols = prog["idx_cols"]
    entry_chunk = prog["entry_chunk"]
    entry_base = prog["entry_base"]
    group_ranges = prog["group_ranges"]
    range_chunk0 = prog["range_chunk0"]
    range_entry0 = prog["range_entry0"]
    seg_of_range = prog["seg_of_range"]
    gpad = prog["gpad"]

    nc = bacc.Bacc(num_swdge_queues=4)
    feat_d = nc.declare_dram_parameter("feat16", [N_NODES, F], mybir.dt.float16, isOutput=False)
    w_d = nc.declare_dram_parameter("w16", [F, F], mybir.dt.float16, isOutput=False)
    bias_d = nc.declare_dram_parameter("biasb", [128, SB], mybir.dt.float32, isOutput=False)
    iota_d = nc.declare_dram_parameter("iota", [128, 128], mybir.dt.float16, isOutput=False)
    idx_d = nc.declare_dram_parameter("idxb", [128, idx_cols], mybir.dt.int16, isOutput=False)
    dl_d = nc.declare_dram_parameter("dlb", [128, prog["total_entries"]], mybir.dt.float16, isOutput=False)
    nd_d = nc.declare_dram_parameter("ndst", [128, NSB * (SB // 128)], mybir.dt.float32, isOutput=False)
    out_d = nc.declare_dram_parameter("out", [NSB * SB, F], mybir.dt.float32, isOutput=True)

    ranges = [(r * RANGE, min((r + 1) * RANGE, N_NODES)) for r in range(NRANGES)]
    max_range_entries = max(range_entry0[r + 1] - range_entry0[r]
                            for r in range(NRANGES))
    max_range_chunks = max(range_chunk0[r + 1] - range_chunk0[r]
                           for r in range(NRANGES))

    with tile.TileContext(nc) as tc:
        with (
            tc.tile_pool(name="const", bufs=1) as constp,
            tc.tile_pool(name="agg", bufs=NSB) as aggp,
            tc.tile_pool(name="et", bufs=4) as etp,
            tc.tile_pool(name="ix", bufs=2) as ixp,
            tc.tile_pool(name="dl", bufs=2) as dlp,
            tc.tile_pool(name="s", bufs=4) as sp,
            tc.tile_pool(name="aggs", bufs=2) as aggsp,
            tc.tile_pool(name="outs", bufs=2) as outsp,
            tc.tile_pool(name="ps", bufs=2, space="PSUM") as psp,
            tc.tile_pool(name="ps2", bufs=2, space="PSUM") as ps2p,
        ):
            w_t = constp.tile([F, F], mybir.dt.float16)
            nc.sync.dma_start(w_t[:], w_d[:])
            bias_t = constp.tile([128, SB], mybir.dt.float32)
            nc.sync.dma_start(bias_t[:], bias_d[:])
            iota_t = constp.tile([128, 128], mybir.dt.float16)
            nc.sync.dma_start(iota_t[:], iota_d[:])
            nd_t = constp.tile([128, NSB * (SB // 128)], mybir.dt.float32)
            nc.sync.dma_start(nd_t[:], nd_d[:])
            zeros_t = constp.tile([128, SB], mybir.dt.float32)
            nc.vector.memset(zeros_t[:], 0.0)

            agg_tiles = [aggp.tile([128, SB], mybir.dt.float32, name=f"agg{s_}", tag="agg")
                         for s_ in range(NSB)]
            init_p = {}
            last_p = {}
            for s in range(NSB):
                ps_ = [p2 for p2 in range(NRANGES)
                       if int(gpad[s, RANGE_ORDER[p2]]) > 0]
                if ps_:
                    init_p[s] = ps_[0]
                    last_p[s] = ps_[-1]

            # calls grouped by (range, segment)
            seg_calls = {}
            for r, g, c0, n in plan:
                seg_calls.setdefault((r, g), []).append((c0, n))

            call_counter = [0]
            # chunk -> (et tile, offset) for the current range
            for p_ in range(NRANGES):
                r = RANGE_ORDER[p_]
                lo, hi = ranges[r]
                rc0 = range_chunk0[p_]
                re0, re1 = range_entry0[p_], range_entry0[p_ + 1]
                ne_r = re1 - re0
                dlt = dlp.tile([128, max_range_entries], mybir.dt.float16, tag="dl")
                nc.sync.dma_start(dlt[:, :ne_r], dl_d[:, re0:re1])
                nch_r = range_chunk0[p_ + 1] - rc0
                ixr = ixp.tile([128, max_range_chunks * 8], mybir.dt.int16, tag="ix")
                nc.sync.dma_start(ixr[:, : nch_r * 8],
                                  idx_d[:, rc0 * 8: (rc0 + nch_r) * 8])

                chunk_tile = {}
                segs_emitted = 0
                groups = [(s,) + group_ranges[(s, r)] for s in range(NSB)
                          if group_ranges[(s, r)][1] > group_ranges[(s, r)][0]]
                gi = 0
                for g in range(seg_of_range[p_]):
                    calls = seg_calls[(r, g)]
                    seg_c0 = calls[0][0]
                    seg_nch = sum(n for _, n in calls) // 128
                    et = etp.tile([128, seg_nch * F], mybir.dt.float16, tag="et")
                    for c0, n in calls:
                        rel = c0 - seg_c0
                        nc.gpsimd.dma_gather(
                            out_ap=et[:, rel * F: (rel + n // 128) * F].rearrange(
                                "p (c e) -> p c e", e=F),
                            in_ap=feat_d[lo:hi, :],
                            idxs_ap=ixr[:, (c0 - rc0) * 8: (c0 - rc0) * 8 + n // 16],
                            num_idxs=n,
                            num_idxs_reg=n,
                            elem_size=F,
                            queue_num=call_counter[0] % 4,
                        )
                        call_counter[0] += 1
                    for c in range(seg_c0, seg_c0 + seg_nch):
                        chunk_tile[c] = (et, c - seg_c0)
                    segs_emitted += 1

                    # emit compute for groups whose last chunk is now gathered
                    last_ready = seg_c0 + seg_nch - 1
                    while gi < len(groups) and groups[gi][2] - 1 <= last_ready:
                        s, gc0, gc1, ge0, ge1 = groups[gi]
                        gi += 1
                        psT = psp.tile([128, SB], mybir.dt.float32, space="PSUM")
                        nc.scalar.copy(psT[:], zeros_t[:])
                        nent = ge1 - ge0
                        for p0 in range(0, nent, PIECE):
                            pe = min(p0 + PIECE, nent)
                            np_ = pe - p0
                            st = sp.tile([128, PIECE * 128], mybir.dt.float16, tag="s")
                            bsl = dlt[:, ge0 - re0 + p0: ge0 - re0 + pe]
                            b = bsl.to_broadcast([128, np_, 128])
                            a = iota_t[:]
                            i2 = AP(a.tensor, a.offset,
                                    [list(a.ap[0]), [0, np_], list(a.ap[1])])
                            nc.vector.tensor_tensor(
                                out=st[:, : np_ * 128].rearrange(
                                    "p (c j) -> p c j", j=128),
                                in0=b, in1=i2, op=mybir.AluOpType.is_equal,
                            )
                            for kk in range(p0, pe):
                                c = int(entry_chunk[ge0 + kk])
                                base = int(entry_base[ge0 + kk])
                                ett, off = chunk_tile[c]
                                nc.tensor.matmul(
                                    out=psT[:, base: base + 128],
                                    lhsT=ett[:, off * F: (off + 1) * F],
                                    rhs=st[:, (kk - p0) * 128: (kk - p0 + 1) * 128],
                                    start=False,
                                    stop=(kk == nent - 1),
                                )
                        if p_ == init_p[s]:
                            nc.scalar.copy(agg_tiles[s][:], psT[:])
                        else:
                            nc.vector.tensor_tensor(
                                out=agg_tiles[s][:], in0=agg_tiles[s][:],
                                in1=psT[:], op=mybir.AluOpType.add,
                            )
                        if p_ == last_p[s]:
                            aggT = aggsp.tile([128, SB], mybir.dt.float16)
                            nc.scalar.copy(aggT[:], agg_tiles[s][:])
                            ps2 = ps2p.tile([128, SB], mybir.dt.float32, space="PSUM")
                            for j in range(SB // F):
                                nc.tensor.matmul(
                                    out=ps2[:, j * F: (j + 1) * F],
                                    lhsT=aggT[:, j * F: (j + 1) * F],
                                    rhs=w_t[:],
                                    start=True,
                                    stop=True,
                                )
                            ot = outsp.tile([128, SB], mybir.dt.float32)
                            for j in range(SB // F):
                                nc.scalar.activation(
                                    ot[:, j * F: (j + 1) * F],
                                    ps2[:, j * F: (j + 1) * F],
                                    mybir.ActivationFunctionType.Copy,
                                    scale=nd_t[:, s * (SB // F) + j:
                                               s * (SB // F) + j + 1],
                                )
                            if with_bias:
                                nc.gpsimd.tensor_tensor(
                                    out=ot[:], in0=ot[:], in1=bias_t[:],
                                    op=mybir.AluOpType.add,
                                )
                            nc.sync.dma_start(
                                out_d[s * SB: (s + 1) * SB, :].rearrange(
                                    "(j p) f -> p j f", p=128),
                                ot[:].rearrange("p (j f) -> p j f", f=F),
                            )
    nc.finalize()
    return nc


def kernel(feat, weight, bias, src, dst):
    _install_walrus_passes()
    from concourse.bass_utils import run_bass_kernel_spmd

    feat = np.asarray(feat, dtype=np.float32)
    weight = np.asarray(weight, dtype=np.float32)
    bias = np.asarray(bias, dtype=np.float32)

    prog, per_core, norm_src, nd_tab = _preprocess(src, dst)
    feat16 = np.ascontiguousarray((feat * norm_src[:, None]).astype(np.float16))
    w16 = np.ascontiguousarray(weight.astype(np.float16))
    with_bias = bool(np.any(bias != 0.0))
    nc = _build_program(prog, with_bias)

    bias_b = np.broadcast_to(np.tile(bias, SB // F)[None, :], (128, SB)).copy()
    iota = np.broadcast_to(np.arange(128, dtype=np.float16)[None, :], (128, 128)).copy()

    in_maps = []
    for k in range(N_CORES):
        idx_buf, dl_buf = per_core[k]
        in_maps.append({
            "feat16": feat16,
            "w16": w16,
            "biasb": bias_b,
            "iota": iota,
            "idxb": idx_buf,
            "dlb": dl_buf,
            "ndst": np.ascontiguousarray(nd_tab[k]),
        })
    res = run_bass_kernel_spmd(nc, in_maps, list(range(N_CORES)))
    out = np.empty((N_CORES * OWN, F), np.float32)
    for k in range(N_CORES):
        out[k * OWN: (k + 1) * OWN] = res.results[k]["out"][:OWN]
    return out[:N_NODES]
